# revision 14
# baseline (speedup 1.0000x reference)
"""Trainium2 Bass kernel for nn_MANN_23965917511952 (8-core SPMD).

Math notes (verified against the jax reference):
  - n = min(B, M) = 128 = M, so the sorted scatter overwrites EVERY memory
    slot: new_mem is just a row-permutation of R_t[:128] from the support
    pass.  softmax(q @ mem^T) @ mem is invariant under row permutation of
    mem, so new_mem can be used in any row order.
  - The updated mem_weight is only consumed by the (discarded) query-pass
    write-back, so argsort/suffix-cumsum/weight softmax are dead code.
Per-core sharding (8 cores):
  - support rows [200c, 200c+200), query rows [220c, 220c+220)  (4 episodes)
  - support rows [16c, 16c+16) (slice of rows 0:128) -> local R16 rows of the
    new memory; 16KB AllGather replicates the full 128x256 new memory.
Output: logits [32,55,11] f32 and pred [1760] int32 (device computes both;
host only concatenates shards).
"""

import os
import sys

for _p in ("/opt/trn_rl_repo",):
    if os.path.isdir(_p) and _p not in sys.path:
        sys.path.insert(0, _p)

import numpy as np

import concourse.bass as bass
import concourse.tile as tile
from concourse import bacc, masks, mybir
from concourse.bass_utils import run_bass_kernel_spmd

F32 = mybir.dt.float32
AOT = mybir.AluOpType
AX = mybir.AxisListType
ACT = mybir.ActivationFunctionType

NCORES = 8
SUP, QRY = 200, 220          # rows per core
L, H, M = 128, 256, 128      # seq len, hidden, memory slots
S16 = 16                     # rows of support[0:128] owned per core
EPC = 4                      # episodes per core
NCLS, QPE, SHOT = 10, 55, 5  # classes, queries/episode, shots
LC = 32                      # l-chunk for the maxpool stream
SUP_BLOCKS = [(0, 128), (128, 200)]
QRY_BLOCKS = [(0, 110), (110, 220)]  # two episodes per block


def _build(nc: bass.Bass):
    sup = nc.dram_tensor("sup", [SUP, L, H], F32, kind="ExternalInput")
    sup1 = nc.dram_tensor("sup1", [S16, L, H], F32, kind="ExternalInput")
    qry = nc.dram_tensor("qry", [QRY, L, H], F32, kind="ExternalInput")
    memt = nc.dram_tensor("memt", [H, M], F32, kind="ExternalInput")
    memb = nc.dram_tensor("memb", [M, H], F32, kind="ExternalInput")
    gamma = nc.dram_tensor("gamma", [H], F32, kind="ExternalInput")
    beta = nc.dram_tensor("beta", [H], F32, kind="ExternalInput")
    poolm = nc.dram_tensor("poolm", [SUP, EPC * NCLS], F32, kind="ExternalInput")
    logits_o = nc.dram_tensor(
        "logits_o", [EPC * QPE, NCLS + 1], F32, kind="ExternalOutput"
    )
    pred_o = nc.dram_tensor("pred_o", [EPC * QPE, 1], mybir.dt.int32, kind="ExternalOutput")

    from contextlib import ExitStack

    with tile.TileContext(nc) as tc, ExitStack() as ctx:
        chunkp = ctx.enter_context(tc.tile_pool(name="chunkp", bufs=3))
        accp = ctx.enter_context(tc.tile_pool(name="accp", bufs=2))
        persist = ctx.enter_context(tc.tile_pool(name="persist", bufs=1))
        work = ctx.enter_context(tc.tile_pool(name="work", bufs=2))
        small = ctx.enter_context(tc.tile_pool(name="small", bufs=4))
        psum2 = ctx.enter_context(tc.tile_pool(name="psum2", bufs=2, space="PSUM"))
        psum1 = ctx.enter_context(tc.tile_pool(name="psum1", bufs=1, space="PSUM"))
        dram = ctx.enter_context(tc.tile_pool(name="dram", bufs=1, space="DRAM"))

        # ---------------- constants ----------------
        ident = persist.tile([128, 128], F32, tag="ident")
        masks.make_identity(nc, ident[:])
        epst = persist.tile([128, 1], F32, tag="epst")
        nc.gpsimd.memset(epst[:], 1e-5)
        ones1 = persist.tile([1, 128], F32, tag="ones1")
        nc.gpsimd.memset(ones1[:], 1.0)

        gab = persist.tile([128, H], F32, tag="gab")
        beb = persist.tile([128, H], F32, tag="beb")
        g_ap = gamma[:]
        nc.gpsimd.dma_start(
            out=gab[:],
            in_=bass.AP(tensor=g_ap.tensor, offset=g_ap.offset, ap=[[0, 128]] + list(g_ap.ap)),
        )
        b_ap = beta[:]
        nc.gpsimd.dma_start(
            out=beb[:],
            in_=bass.AP(tensor=b_ap.tensor, offset=b_ap.offset, ap=[[0, 128]] + list(b_ap.ap)),
        )
        memT0 = persist.tile([128, M], F32, tag="memT0")
        memT1 = persist.tile([128, M], F32, tag="memT1")
        nc.gpsimd.dma_start(out=memT0[:], in_=memt[0:128, :])
        nc.gpsimd.dma_start(out=memT1[:], in_=memt[128:256, :])
        memb_sb = persist.tile([M, H], F32, tag="membsb")
        nc.gpsimd.dma_start(out=memb_sb[:], in_=memb[:, :])
        poolm0 = persist.tile([128, EPC * NCLS], F32, tag="poolm0")
        poolm1 = persist.tile([SUP - 128, EPC * NCLS], F32, tag="poolm1")
        nc.gpsimd.dma_start(out=poolm0[:], in_=poolm[0:128, :])
        nc.gpsimd.dma_start(out=poolm1[:], in_=poolm[128:SUP, :])

        # ---------------- helpers ----------------
        def layernorm(x, rows):
            stats = small.tile([128, 6], F32, tag="stats")
            mv = small.tile([128, 2], F32, tag="mv")
            nc.vector.bn_stats(out=stats[:rows], in_=x[:rows, :])
            nc.vector.bn_aggr(out=mv[:rows], in_=stats[:rows])
            rstd = small.tile([128, 1], F32, tag="rstd")
            nc.scalar.activation(
                out=rstd[:rows], in_=mv[:rows, 1:2], func=ACT.Sqrt,
                bias=epst[:rows], scale=1.0,
            )
            nc.vector.reciprocal(out=rstd[:rows], in_=rstd[:rows])
            nc.vector.tensor_scalar(
                out=x[:rows, :], in0=x[:rows, :],
                scalar1=mv[:rows, 0:1], scalar2=rstd[:rows],
                op0=AOT.subtract, op1=AOT.mult,
            )
            nc.vector.tensor_mul(x[:rows, :], x[:rows, :], gab[:rows, :])
            nc.vector.tensor_add(x[:rows, :], x[:rows, :], beb[:rows, :])

        def mann_core(x, rows, rhs0, rhs1, bank):
            """R = softmax(x @ bank^T) @ bank; returns R in PSUM [rows, H]."""
            xT = []
            for k in range(2):
                tp = psum2.tile([128, 128], F32, tag="tr")
                nc.tensor.transpose(
                    tp[:, :rows], x[:rows, k * 128:(k + 1) * 128], ident[:rows, :rows]
                )
                xk = work.tile([128, 128], F32, tag=f"xT{k}")
                nc.scalar.copy(out=xk[:, :rows], in_=tp[:, :rows])
                xT.append(xk)
            G = psum1.tile([128, M], F32, tag="G")
            nc.tensor.matmul(G[:rows, :], xT[0][:, :rows], rhs0[:, :], start=True, stop=False)
            nc.tensor.matmul(G[:rows, :], xT[1][:, :rows], rhs1[:, :], start=False, stop=True)
            nmax = small.tile([128, 1], F32, tag="nmax")
            nc.vector.tensor_reduce(
                out=nmax[:rows], in_=G[:rows, :], op=AOT.max, axis=AX.X, negate=True
            )
            W = work.tile([128, M], F32, tag="W")
            ssum = small.tile([128, 1], F32, tag="ssum")
            nc.scalar.activation(
                out=W[:rows, :], in_=G[:rows, :], func=ACT.Exp,
                bias=nmax[:rows], scale=1.0, accum_out=ssum[:rows],
            )
            nc.vector.reciprocal(out=ssum[:rows], in_=ssum[:rows])
            nc.vector.tensor_scalar_mul(out=W[:rows, :], in0=W[:rows, :], scalar1=ssum[:rows])
            tpw = psum2.tile([128, 128], F32, tag="tr")
            nc.tensor.transpose(tpw[:, :rows], W[:rows, :], ident[:rows, :rows])
            WT = work.tile([128, 128], F32, tag="WT")
            nc.scalar.copy(out=WT[:, :rows], in_=tpw[:, :rows])
            R = psum2.tile([128, H], F32, tag="R")
            nc.tensor.matmul(R[:rows, :], WT[:, :rows], bank[:, :], start=True, stop=True)
            return R

        def maxpool_block(src, r0, rows, dst):
            """dst[:rows] = max over L of src[r0:r0+rows]; streams L in chunks."""
            half = LC // 2   # 16
            acc = accp.tile([128, half, H], F32, tag="acc")
            for lc in range(L // LC):
                ch = chunkp.tile([128, LC, H], F32, tag="chunk")
                nc.sync.dma_start(
                    out=ch[:rows], in_=src[r0:r0 + rows, lc * LC:(lc + 1) * LC, :]
                )
                if lc == 0:
                    nc.vector.tensor_max(
                        acc[:rows], ch[:rows, :half, :], ch[:rows, half:, :]
                    )
                else:
                    nc.vector.tensor_max(
                        ch[:rows, :half, :], ch[:rows, :half, :], ch[:rows, half:, :]
                    )
                    nc.vector.tensor_max(acc[:rows], acc[:rows], ch[:rows, :half, :])
            w = half
            while w > 2:
                nc.vector.tensor_max(
                    acc[:rows, : w // 2, :], acc[:rows, : w // 2, :],
                    acc[:rows, w // 2: w, :],
                )
                w //= 2
            nc.vector.tensor_max(dst[:rows, :], acc[:rows, 0, :], acc[:rows, 1, :])

        # ---------------- sup1 -> new memory (starts first) ----------------
        s1 = chunkp.tile([128, S16, H], F32, tag="chunk")
        ap1 = sup1[:, :, :]
        s1src = bass.AP(
            tensor=ap1.tensor, offset=ap1.offset,
            ap=[[L * H // 8, 128], [H, S16], [1, H]],
        )
        nc.sync.dma_start(out=s1[:, :, :], in_=s1src)
        w = S16
        while w > 2:
            nc.vector.tensor_max(s1[:, : w // 2, :], s1[:, : w // 2, :], s1[:, w // 2: w, :])
            w //= 2
        t256 = work.tile([128, H], F32, tag="t256")
        nc.vector.tensor_max(t256[:, :], s1[:, 0, :], s1[:, 1, :])
        # partition regroup (r a) -> r via a DRAM bounce
        scr = dram.tile([128, H], F32, tag="scr")
        nc.gpsimd.dma_start(out=scr[:, :], in_=t256[:, :])
        s1b = work.tile([S16, 8, H], F32, tag="s1b")
        nc.gpsimd.dma_start(
            out=s1b[:, :, :], in_=scr[:, :].rearrange("(r a) h -> r a h", a=8)
        )
        w = 8
        while w > 2:
            nc.vector.tensor_max(
                s1b[:, : w // 2, :], s1b[:, : w // 2, :], s1b[:, w // 2: w, :]
            )
            w //= 2
        s16t = work.tile([S16, H], F32, tag="s16t")
        nc.vector.tensor_max(s16t[:, :], s1b[:, 0, :], s1b[:, 1, :])
        layernorm(s16t, S16)
        R16 = mann_core(s16t, S16, memT0, memT1, memb_sb)
        r16sb = work.tile([S16, H], F32, tag="r16sb")
        nc.scalar.copy(out=r16sb[:, :], in_=R16[:S16, :])
        cc_in = dram.tile([S16, H], F32, tag="ccin")
        cc_out = dram.tile([M, H], F32, tag="ccout")
        nc.gpsimd.dma_start(out=cc_in[:, :], in_=r16sb[:, :])
        nc.gpsimd.collective_compute(
            "AllGather",
            AOT.bypass,
            replica_groups=[list(range(NCORES))],
            ins=[cc_in.opt()],
            outs=[cc_out.opt()],
        )
        newmem = persist.tile([M, H], F32, tag="newmem")
        nc.gpsimd.dma_start(out=newmem[:, :], in_=cc_out[:, :])
        nmT = []
        for k in range(2):
            tp = psum2.tile([128, 128], F32, tag="tr")
            nc.tensor.transpose(tp[:, :], newmem[:, k * 128:(k + 1) * 128], ident[:, :])
            nk = persist.tile([128, 128], F32, tag=f"nmT{k}")
            nc.scalar.copy(out=nk[:, :], in_=tp[:, :])
            nmT.append(nk)

        # ---------------- support stream ----------------
        proto = psum1.tile([EPC * NCLS, H], F32, tag="proto")
        for b, (r0, r1) in enumerate(SUP_BLOCKS):
            rows = r1 - r0
            sn = persist.tile([128, H], F32, tag=f"sn{b}")
            maxpool_block(sup, r0, rows, sn)
            layernorm(sn, rows)
            R = mann_core(sn, rows, memT0, memT1, memb_sb)
            nc.vector.tensor_add(sn[:rows, :], R[:rows, :], sn[:rows, :])
            pm = poolm0 if b == 0 else poolm1
            nc.tensor.matmul(
                proto[:, :], pm[:rows, :], sn[:rows, :],
                start=(b == 0), stop=(b == len(SUP_BLOCKS) - 1),
            )

        proto_sb = persist.tile([EPC * NCLS, H], F32, tag="protosb")
        nc.scalar.copy(out=proto_sb[:, :], in_=proto[:, :])
        p2 = persist.tile([EPC * NCLS, 1], F32, tag="p2")
        psq = work.tile([EPC * NCLS, H], F32, tag="psq")
        nc.scalar.activation(
            out=psq[:, :], in_=proto_sb[:, :], func=ACT.Square, accum_out=p2[:, :]
        )
        NP = EPC * NCLS  # 40
        pT = []
        for k in range(2):
            tp = psum2.tile([128, 128], F32, tag="tr")
            nc.tensor.transpose(
                tp[:, :NP], proto_sb[:NP, k * 128:(k + 1) * 128], ident[:NP, :NP]
            )
            pk = persist.tile([128, NP], F32, tag=f"pT{k}")
            nc.scalar.mul(out=pk[:, :], in_=tp[:, :NP], mul=2.0)
            pT.append(pk)
        tpp = psum2.tile([128, 128], F32, tag="tr")
        nc.tensor.transpose(tpp[:1, :NP], p2[:NP, 0:1], ident[:NP, :NP])
        negp2 = persist.tile([1, NP], F32, tag="negp2")
        nc.scalar.mul(out=negp2[:, :], in_=tpp[:1, :NP], mul=-1.0)

        # ---------------- query stream + episodes ----------------
        def episode_out(qv, q2e, ep):
            qT = []
            for k in range(2):
                tp = psum2.tile([128, 128], F32, tag="tr")
                nc.tensor.transpose(
                    tp[:, :QPE], qv[:QPE, k * 128:(k + 1) * 128],
                    ident[:QPE, :QPE],
                )
                qk = work.tile([128, QPE], F32, tag=f"qT{k}")
                nc.scalar.copy(out=qk[:, :], in_=tp[:, :QPE])
                qT.append(qk)
            dot = psum1.tile([QPE, 16], F32, tag="dot")
            nc.tensor.matmul(
                dot[:, :NCLS], qT[0][:, :], pT[0][:, ep * NCLS:(ep + 1) * NCLS],
                start=True, stop=False,
            )
            nc.tensor.matmul(
                dot[:, :NCLS], qT[1][:, :], pT[1][:, ep * NCLS:(ep + 1) * NCLS],
                start=False, stop=False,
            )
            nc.tensor.matmul(
                dot[:, :NCLS], ones1[:1, :QPE], negp2[:1, ep * NCLS:(ep + 1) * NCLS],
                start=False, stop=True,
            )
            lf = work.tile([QPE, 16], F32, tag="lf")
            nc.vector.tensor_scalar(
                out=lf[:, 0:NCLS], in0=dot[:, :NCLS],
                scalar1=q2e[:QPE], scalar2=None, op0=AOT.subtract,
            )
            mn = small.tile([QPE, 1], F32, tag="mn")
            nc.vector.tensor_reduce(out=mn[:, :], in_=lf[:, 0:NCLS], op=AOT.min, axis=AX.X)
            nc.vector.tensor_scalar_add(out=lf[:, NCLS:NCLS + 1], in0=mn[:, :], scalar1=-1.0)
            mx8 = small.tile([QPE, 8], F32, tag="mx8")
            nc.vector.max(out=mx8[:, :], in_=lf[:, 0:NCLS + 1])
            idx8 = small.tile([QPE, 8], mybir.dt.uint32, tag="idx8")
            nc.vector.max_index(out=idx8[:, :], in_max=mx8[:, :], in_values=lf[:, 0:NCLS + 1])
            nc.gpsimd.dma_start(
                out=logits_o[ep * QPE:(ep + 1) * QPE, :], in_=lf[:, 0:NCLS + 1]
            )
            nc.gpsimd.dma_start(
                out=pred_o[ep * QPE:(ep + 1) * QPE, :],
                in_=idx8[:, 0:1].bitcast(mybir.dt.int32),
            )

        for qb, (r0, r1) in enumerate(QRY_BLOCKS):
            rows = r1 - r0
            qn = persist.tile([128, H], F32, tag=f"qn{qb}")
            maxpool_block(qry, r0, rows, qn)
            layernorm(qn, rows)
            R = mann_core(qn, rows, nmT[0], nmT[1], newmem)
            nc.vector.tensor_add(qn[:rows, :], R[:rows, :], qn[:rows, :])
            for e_loc in range(2):
                if e_loc == 0:
                    qv = qn
                else:
                    # rebase episode rows 55:110 to partition 0 (PE operands
                    # must start at partition 0/32/64) via SBUF->SBUF DMA
                    qsc = work.tile([QPE, H], F32, tag="qsc")
                    nc.gpsimd.dma_start(out=qsc[:, :], in_=qn[QPE:rows, :])
                    qv = qsc
                q2e = small.tile([QPE, 1], F32, tag="q2e")
                qsq = work.tile([QPE, H], F32, tag="qsq")
                nc.scalar.activation(
                    out=qsq[:, :], in_=qv[:QPE, :], func=ACT.Square,
                    accum_out=q2e[:, :],
                )
                episode_out(qv, q2e, qb * 2 + e_loc)

    return nc


_BUILT = None


def _get_nc():
    global _BUILT
    if _BUILT is None:
        nc = bacc.Bacc(num_devices=NCORES)
        _build(nc)
        if not nc.is_finalized():
            nc.finalize()
        _BUILT = nc
    return _BUILT


def _pool_matrix():
    pm = np.zeros((SUP, EPC * NCLS), np.float32)
    for g in range(EPC * NCLS):
        pm[g * SHOT:(g + 1) * SHOT, g] = 1.0 / SHOT
    return pm


def _shard_inputs(support, query, gamma, beta, mem):
    memT = np.ascontiguousarray(mem.T)
    pm = _pool_matrix()
    gam = np.ascontiguousarray(gamma)
    bet = np.ascontiguousarray(beta)
    in_maps = []
    for c in range(NCORES):
        in_maps.append(
            {
                "sup": np.ascontiguousarray(support[c * SUP:(c + 1) * SUP]),
                "sup1": np.ascontiguousarray(support[c * S16:(c + 1) * S16]),
                "qry": np.ascontiguousarray(query[c * QRY:(c + 1) * QRY]),
                "memt": memT,
                "memb": np.ascontiguousarray(mem),
                "gamma": gam,
                "beta": bet,
                "poolm": pm,
            }
        )
    return in_maps


def run(inputs, trace=False, trace_cores=None):
    support = np.asarray(inputs["support"], dtype=np.float32)
    query = np.asarray(inputs["query"], dtype=np.float32)
    gamma = np.asarray(inputs["gamma"], dtype=np.float32)
    beta = np.asarray(inputs["beta"], dtype=np.float32)
    mem = np.asarray(inputs["mem"], dtype=np.float32)
    assert support.shape == (NCORES * SUP, L, H), support.shape
    assert query.shape == (NCORES * QRY, L, H), query.shape
    nc = _get_nc()
    in_maps = _shard_inputs(support, query, gamma, beta, mem)
    res = run_bass_kernel_spmd(
        nc, in_maps, list(range(NCORES)), trace=trace, trace_cores=trace_cores
    )
    logits = np.concatenate(
        [res.results[c]["logits_o"].reshape(EPC, QPE, NCLS + 1) for c in range(NCORES)],
        axis=0,
    )
    pred = np.concatenate(
        [res.results[c]["pred_o"].reshape(-1) for c in range(NCORES)]
    ).astype(np.int32)
    return (logits, pred), res


def kernel(**inputs):
    (logits, pred), _ = run(inputs)
    return logits, pred


# revision 17
# speedup vs baseline: 1.1839x; 1.1839x over previous
"""Trainium2 Bass kernel for nn_MANN_23965917511952 (8-core SPMD).

Math notes (verified against the jax reference):
  - n = min(B, M) = 128 = M, so the sorted scatter overwrites EVERY memory
    slot: new_mem is just a row-permutation of R_t[:128] from the support
    pass.  softmax(q @ mem^T) @ mem is invariant under row permutation of
    mem, so new_mem can be used in any row order.
  - The updated mem_weight is only consumed by the (discarded) query-pass
    write-back, so argsort/suffix-cumsum/weight softmax are dead code.

Per-core sharding (8 cores):
  - support rows [200c, 200c+200), query rows [220c, 220c+220)  (4 episodes)
  - support rows [16c, 16c+16) (slice of rows 0:128) -> local R16 rows of the
    new memory; a 16KB AllGather replicates the full 128x256 new memory.

Maxpool streaming layout: every big load fills all 128 SBUF partitions
(partition-starved DMAs measured ~1.6x slower).  A "unit" is one (row,
L-half) pair = 64*256 floats = 64KB contiguous in DRAM; a full load is 128
consecutive units as [128, 64, 256] with 64KB-per-partition descriptors.
The in-tile max tree collapses each load to per-unit maxima [128, 256];
per-row maxima are then regrouped via a small DRAM bounce (partition-pair
reduction is impossible on the per-lane compute engines).

Output: logits [32,55,11] f32 and pred [1760] int32 (device computes both;
host only concatenates shards).
"""

import os
import sys

for _p in ("/opt/trn_rl_repo",):
    if os.path.isdir(_p) and _p not in sys.path:
        sys.path.insert(0, _p)

import numpy as np

import concourse.bass as bass
import concourse.tile as tile
from concourse import bacc, masks, mybir
from concourse.bass_utils import run_bass_kernel_spmd

F32 = mybir.dt.float32
AOT = mybir.AluOpType
AX = mybir.AxisListType
ACT = mybir.ActivationFunctionType

NCORES = 8
SUP, QRY = 200, 220          # rows per core
L, H, M = 128, 256, 128      # seq len, hidden, memory slots
S16 = 16                     # rows of support[0:128] owned per core
EPC = 4                      # episodes per core
NCLS, QPE, SHOT = 10, 55, 5  # classes, queries/episode, shots
UNIT = (L // 2) * H          # one (row, L-half) unit: 16384 f32 = 64KB
SUP_BLOCKS = [(0, 128), (128, 200)]
QRY_BLOCKS = [(0, 128), (128, 220)]


def _build(nc: bass.Bass):
    sup = nc.dram_tensor("sup", [SUP, L, H], F32, kind="ExternalInput")
    sup1 = nc.dram_tensor("sup1", [S16, L, H], F32, kind="ExternalInput")
    qry = nc.dram_tensor("qry", [QRY, L, H], F32, kind="ExternalInput")
    memt = nc.dram_tensor("memt", [H, M], F32, kind="ExternalInput")
    memb = nc.dram_tensor("memb", [M, H], F32, kind="ExternalInput")
    gamma = nc.dram_tensor("gamma", [H], F32, kind="ExternalInput")
    beta = nc.dram_tensor("beta", [H], F32, kind="ExternalInput")
    poolm = nc.dram_tensor("poolm", [SUP, EPC * NCLS], F32, kind="ExternalInput")
    logits_o = nc.dram_tensor(
        "logits_o", [EPC * QPE, NCLS + 1], F32, kind="ExternalOutput"
    )
    pred_o = nc.dram_tensor("pred_o", [EPC * QPE, 1], mybir.dt.int32, kind="ExternalOutput")

    from contextlib import ExitStack

    with tile.TileContext(nc) as tc, ExitStack() as ctx:
        chunkp = ctx.enter_context(tc.tile_pool(name="chunkp", bufs=2))
        persist = ctx.enter_context(tc.tile_pool(name="persist", bufs=1))
        single = ctx.enter_context(tc.tile_pool(name="single", bufs=1))
        work = ctx.enter_context(tc.tile_pool(name="work", bufs=2))
        small = ctx.enter_context(tc.tile_pool(name="small", bufs=4))
        psum2 = ctx.enter_context(tc.tile_pool(name="psum2", bufs=2, space="PSUM"))
        psum1 = ctx.enter_context(tc.tile_pool(name="psum1", bufs=1, space="PSUM"))
        dram = ctx.enter_context(tc.tile_pool(name="dram", bufs=1, space="DRAM"))

        # ---------------- constants ----------------
        ident = persist.tile([128, 128], F32, tag="ident")
        masks.make_identity(nc, ident[:])
        epst = persist.tile([128, 1], F32, tag="epst")
        nc.gpsimd.memset(epst[:], 1e-5)
        ones1 = persist.tile([1, 128], F32, tag="ones1")
        nc.gpsimd.memset(ones1[:], 1.0)

        gab = persist.tile([128, H], F32, tag="gab")
        beb = persist.tile([128, H], F32, tag="beb")
        g_ap = gamma[:]
        nc.gpsimd.dma_start(
            out=gab[:],
            in_=bass.AP(tensor=g_ap.tensor, offset=g_ap.offset, ap=[[0, 128]] + list(g_ap.ap)),
        )
        b_ap = beta[:]
        nc.gpsimd.dma_start(
            out=beb[:],
            in_=bass.AP(tensor=b_ap.tensor, offset=b_ap.offset, ap=[[0, 128]] + list(b_ap.ap)),
        )
        memT0 = persist.tile([128, M], F32, tag="memT0")
        memT1 = persist.tile([128, M], F32, tag="memT1")
        nc.gpsimd.dma_start(out=memT0[:], in_=memt[0:128, :])
        nc.gpsimd.dma_start(out=memT1[:], in_=memt[128:256, :])
        memb_sb = persist.tile([M, H], F32, tag="membsb")
        nc.gpsimd.dma_start(out=memb_sb[:], in_=memb[:, :])
        poolm0 = persist.tile([128, EPC * NCLS], F32, tag="poolm0")
        poolm1 = persist.tile([SUP - 128, EPC * NCLS], F32, tag="poolm1")
        nc.gpsimd.dma_start(out=poolm0[:], in_=poolm[0:128, :])
        nc.gpsimd.dma_start(out=poolm1[:], in_=poolm[128:SUP, :])

        # ---------------- helpers ----------------
        def tree_max(t, parts, width):
            """In-place halving max over axis 1 of t[:parts, :width, :]."""
            w = width
            while w > 1:
                nc.vector.tensor_max(
                    t[:parts, : w // 2, :], t[:parts, : w // 2, :], t[:parts, w // 2: w, :]
                )
                w //= 2

        def layernorm(x, rows):
            stats = small.tile([128, 6], F32, tag="stats")
            mv = small.tile([128, 2], F32, tag="mv")
            nc.vector.bn_stats(out=stats[:rows], in_=x[:rows, :])
            nc.vector.bn_aggr(out=mv[:rows], in_=stats[:rows])
            rstd = small.tile([128, 1], F32, tag="rstd")
            nc.scalar.activation(
                out=rstd[:rows], in_=mv[:rows, 1:2], func=ACT.Sqrt,
                bias=epst[:rows], scale=1.0,
            )
            nc.vector.reciprocal(out=rstd[:rows], in_=rstd[:rows])
            nc.vector.tensor_scalar(
                out=x[:rows, :], in0=x[:rows, :],
                scalar1=mv[:rows, 0:1], scalar2=rstd[:rows],
                op0=AOT.subtract, op1=AOT.mult,
            )
            nc.vector.tensor_mul(x[:rows, :], x[:rows, :], gab[:rows, :])
            nc.vector.tensor_add(x[:rows, :], x[:rows, :], beb[:rows, :])

        def mann_core(x, rows, rhs0, rhs1, bank):
            """R = softmax(x @ bank^T) @ bank; returns R in PSUM [rows, H]."""
            xT = []
            for k in range(2):
                tp = psum2.tile([128, 128], F32, tag="tr")
                nc.tensor.transpose(
                    tp[:, :rows], x[:rows, k * 128:(k + 1) * 128], ident[:rows, :rows]
                )
                xk = work.tile([128, 128], F32, tag=f"xT{k}")
                nc.scalar.copy(out=xk[:, :rows], in_=tp[:, :rows])
                xT.append(xk)
            G = psum1.tile([128, M], F32, tag="G")
            nc.tensor.matmul(G[:rows, :], xT[0][:, :rows], rhs0[:, :], start=True, stop=False)
            nc.tensor.matmul(G[:rows, :], xT[1][:, :rows], rhs1[:, :], start=False, stop=True)
            nmax = small.tile([128, 1], F32, tag="nmax")
            nc.vector.tensor_reduce(
                out=nmax[:rows], in_=G[:rows, :], op=AOT.max, axis=AX.X, negate=True
            )
            W = work.tile([128, M], F32, tag="W")
            ssum = small.tile([128, 1], F32, tag="ssum")
            nc.scalar.activation(
                out=W[:rows, :], in_=G[:rows, :], func=ACT.Exp,
                bias=nmax[:rows], scale=1.0, accum_out=ssum[:rows],
            )
            nc.vector.reciprocal(out=ssum[:rows], in_=ssum[:rows])
            nc.vector.tensor_scalar_mul(out=W[:rows, :], in0=W[:rows, :], scalar1=ssum[:rows])
            tpw = psum2.tile([128, 128], F32, tag="tr")
            nc.tensor.transpose(tpw[:, :rows], W[:rows, :], ident[:rows, :rows])
            WT = work.tile([128, 128], F32, tag="WT")
            nc.scalar.copy(out=WT[:, :rows], in_=tpw[:, :rows])
            R = psum2.tile([128, H], F32, tag="R")
            nc.tensor.matmul(R[:rows, :], WT[:, :rows], bank[:, :], start=True, stop=True)
            return R

        # ---------------- sup1 -> new memory (starts first) ----------------
        s1 = chunkp.tile([128, S16, H], F32, tag="chunk")
        ap1 = sup1[:, :, :]
        s1src = bass.AP(
            tensor=ap1.tensor, offset=ap1.offset,
            ap=[[L * H // 8, 128], [H, S16], [1, H]],
        )
        nc.sync.dma_start(out=s1[:, :, :], in_=s1src)
        tree_max(s1, 128, S16)
        # partition regroup (r a) -> r via a DRAM bounce
        scr1 = dram.tile([128, H], F32, tag="scr1")
        nc.gpsimd.dma_start(out=scr1[:, :], in_=s1[:, 0, :])
        s1b = single.tile([S16, 8, H], F32, tag="s1b")
        nc.gpsimd.dma_start(
            out=s1b[:, :, :], in_=scr1[:, :].rearrange("(r a) h -> r a h", a=8)
        )
        tree_max(s1b, S16, 8)
        s16t = single.tile([S16, H], F32, tag="s16t")
        nc.vector.tensor_copy(s16t[:, :], s1b[:, 0, :])
        layernorm(s16t, S16)
        R16 = mann_core(s16t, S16, memT0, memT1, memb_sb)
        r16sb = single.tile([S16, H], F32, tag="r16sb")
        nc.scalar.copy(out=r16sb[:, :], in_=R16[:S16, :])
        cc_in = dram.tile([S16, H], F32, tag="ccin")
        cc_out = dram.tile([M, H], F32, tag="ccout")
        nc.gpsimd.dma_start(out=cc_in[:, :], in_=r16sb[:, :])
        nc.gpsimd.collective_compute(
            "AllGather",
            AOT.bypass,
            replica_groups=[list(range(NCORES))],
            ins=[cc_in.opt()],
            outs=[cc_out.opt()],
        )
        newmem = persist.tile([M, H], F32, tag="newmem")
        nc.gpsimd.dma_start(out=newmem[:, :], in_=cc_out[:, :])
        nmT = []
        for k in range(2):
            tp = psum2.tile([128, 128], F32, tag="tr")
            nc.tensor.transpose(tp[:, :], newmem[:, k * 128:(k + 1) * 128], ident[:, :])
            nk = persist.tile([128, 128], F32, tag=f"nmT{k}")
            nc.scalar.copy(out=nk[:, :], in_=tp[:, :])
            nmT.append(nk)

        # ---------------- streaming maxpool (unit-packed full loads) -------
        # scr_* hold per-unit maxima: unit u = (row u//2, L-half u%2).
        scr_sup = dram.tile([384, H], F32, tag="scr_sup")
        scr_sup_t = dram.tile([32, H], F32, tag="scr_sup_t")    # 8 rows x G=4
        scr_qry = dram.tile([384, H], F32, tag="scr_qry")
        scr_qry_t = dram.tile([112, H], F32, tag="scr_qry_t")   # 28 rows x G=4

        def stream_full(src_handle, j, scr):
            """Load units [128j, 128j+128) as [128, 64, H], reduce, park."""
            t = chunkp.tile([128, L // 2, H], F32, tag="chunk")
            a = src_handle[:, :, :]
            src = bass.AP(
                tensor=a.tensor, offset=a.offset + j * 128 * UNIT,
                ap=[[UNIT, 128], [H, L // 2], [1, H]],
            )
            nc.sync.dma_start(out=t[:, :, :], in_=src)
            tree_max(t, 128, L // 2)
            nc.gpsimd.dma_start(out=scr[j * 128:(j + 1) * 128, :], in_=t[:, 0, :])

        def stream_tail(src_handle, row0, rows, g, scr):
            """Tail rows packed (r, lh=g) -> rows*g partitions, L//g slices."""
            parts = rows * g
            ll = L // g
            t = chunkp.tile([128, L // 2, H], F32, tag="chunk")
            a = src_handle[:, :, :]
            src = bass.AP(
                tensor=a.tensor, offset=a.offset + row0 * L * H,
                ap=[[ll * H, parts], [H, ll], [1, H]],
            )
            nc.sync.dma_start(out=t[:parts, :ll, :], in_=src)
            w = ll
            while w > 1:
                nc.vector.tensor_max(
                    t[:parts, : w // 2, :], t[:parts, : w // 2, :], t[:parts, w // 2: w, :]
                )
                w //= 2
            nc.gpsimd.dma_start(out=scr[0:parts, :], in_=t[:parts, 0, :])

        for j in range(3):
            stream_full(sup, j, scr_sup)
        stream_tail(sup, 192, 8, 4, scr_sup_t)
        for j in range(3):
            stream_full(qry, j, scr_qry)
        stream_tail(qry, 192, 28, 4, scr_qry_t)

        def combine_full(scr, u0, rows_out, dst):
            """dst[:rows_out] = max over the 2 halves; units u0:u0+2*rows."""
            sb = single.tile([128, 2, H], F32, tag="comb")
            nc.gpsimd.dma_start(
                out=sb[:rows_out, :, :],
                in_=scr[u0:u0 + 2 * rows_out, :].rearrange("(r g) h -> r g h", g=2),
            )
            nc.vector.tensor_max(dst[:rows_out, :], sb[:rows_out, 0, :], sb[:rows_out, 1, :])

        def combine_tail(scr, rows, g, dst, dst_off):
            sb = single.tile([28, 4, H], F32, tag="combt")
            nc.gpsimd.dma_start(
                out=sb[:rows, :g, :],
                in_=scr[0:rows * g, :].rearrange("(r g) h -> r g h", g=g),
            )
            w = g
            while w > 1:
                nc.vector.tensor_max(
                    sb[:rows, : w // 2, :], sb[:rows, : w // 2, :], sb[:rows, w // 2: w, :]
                )
                w //= 2
            # rebase to dst rows [dst_off, dst_off+rows) (partition shift)
            nc.gpsimd.dma_start(
                out=dst[dst_off:dst_off + rows, :], in_=sb[:rows, 0, :]
            )

        # ---------------- support blocks ----------------
        proto = psum1.tile([EPC * NCLS, H], F32, tag="proto")
        sn = []
        for b, (r0, r1) in enumerate(SUP_BLOCKS):
            rows = r1 - r0
            x = persist.tile([128, H], F32, tag=f"sn{b}")
            sn.append(x)
            if b == 0:
                combine_full(scr_sup, 0, 128, x)
            else:
                combine_full(scr_sup, 256, 64, x)
                combine_tail(scr_sup_t, 8, 4, x, 64)
            layernorm(x, rows)
            R = mann_core(x, rows, memT0, memT1, memb_sb)
            nc.vector.tensor_add(x[:rows, :], R[:rows, :], x[:rows, :])
            pm = poolm0 if b == 0 else poolm1
            nc.tensor.matmul(
                proto[:, :], pm[:rows, :], x[:rows, :],
                start=(b == 0), stop=(b == len(SUP_BLOCKS) - 1),
            )

        NP = EPC * NCLS  # 40
        proto_sb = persist.tile([NP, H], F32, tag="protosb")
        nc.scalar.copy(out=proto_sb[:, :], in_=proto[:, :])
        p2 = persist.tile([NP, 1], F32, tag="p2")
        psq = single.tile([NP, H], F32, tag="psq")
        nc.scalar.activation(
            out=psq[:, :], in_=proto_sb[:, :], func=ACT.Square, accum_out=p2[:, :]
        )
        pT = []
        for k in range(2):
            tp = psum2.tile([128, 128], F32, tag="tr")
            nc.tensor.transpose(
                tp[:, :NP], proto_sb[:NP, k * 128:(k + 1) * 128], ident[:NP, :NP]
            )
            pk = persist.tile([128, NP], F32, tag=f"pT{k}")
            nc.scalar.mul(out=pk[:, :], in_=tp[:, :NP], mul=2.0)
            pT.append(pk)
        tpp = psum2.tile([128, 128], F32, tag="tr")
        nc.tensor.transpose(tpp[:1, :NP], p2[:NP, 0:1], ident[:NP, :NP])
        negp2 = persist.tile([1, NP], F32, tag="negp2")
        nc.scalar.mul(out=negp2[:, :], in_=tpp[:1, :NP], mul=-1.0)

        # ---------------- query blocks + episodes ----------------
        def episode_out(qv, q2e, ep):
            qT = []
            for k in range(2):
                tp = psum2.tile([128, 128], F32, tag="tr")
                nc.tensor.transpose(
                    tp[:, :QPE], qv[:QPE, k * 128:(k + 1) * 128],
                    ident[:QPE, :QPE],
                )
                qk = work.tile([128, QPE], F32, tag=f"qT{k}")
                nc.scalar.copy(out=qk[:, :], in_=tp[:, :QPE])
                qT.append(qk)
            dot = psum1.tile([QPE, 16], F32, tag="dot")
            nc.tensor.matmul(
                dot[:, :NCLS], qT[0][:, :], pT[0][:, ep * NCLS:(ep + 1) * NCLS],
                start=True, stop=False,
            )
            nc.tensor.matmul(
                dot[:, :NCLS], qT[1][:, :], pT[1][:, ep * NCLS:(ep + 1) * NCLS],
                start=False, stop=False,
            )
            nc.tensor.matmul(
                dot[:, :NCLS], ones1[:1, :QPE], negp2[:1, ep * NCLS:(ep + 1) * NCLS],
                start=False, stop=True,
            )
            lf = work.tile([QPE, 16], F32, tag="lf")
            nc.vector.tensor_scalar(
                out=lf[:, 0:NCLS], in0=dot[:, :NCLS],
                scalar1=q2e[:QPE], scalar2=None, op0=AOT.subtract,
            )
            mn = small.tile([QPE, 1], F32, tag="mn")
            nc.vector.tensor_reduce(out=mn[:, :], in_=lf[:, 0:NCLS], op=AOT.min, axis=AX.X)
            nc.vector.tensor_scalar_add(out=lf[:, NCLS:NCLS + 1], in0=mn[:, :], scalar1=-1.0)
            mx8 = small.tile([QPE, 8], F32, tag="mx8")
            nc.vector.max(out=mx8[:, :], in_=lf[:, 0:NCLS + 1])
            idx8 = small.tile([QPE, 8], mybir.dt.uint32, tag="idx8")
            nc.vector.max_index(out=idx8[:, :], in_max=mx8[:, :], in_values=lf[:, 0:NCLS + 1])
            nc.gpsimd.dma_start(
                out=logits_o[ep * QPE:(ep + 1) * QPE, :], in_=lf[:, 0:NCLS + 1]
            )
            nc.gpsimd.dma_start(
                out=pred_o[ep * QPE:(ep + 1) * QPE, :],
                in_=idx8[:, 0:1].bitcast(mybir.dt.int32),
            )

        qn = []
        for qb, (r0, r1) in enumerate(QRY_BLOCKS):
            rows = r1 - r0
            x = persist.tile([128, H], F32, tag=f"qn{qb}")
            qn.append(x)
            if qb == 0:
                combine_full(scr_qry, 0, 128, x)
            else:
                combine_full(scr_qry, 256, 64, x)
                combine_tail(scr_qry_t, 28, 4, x, 64)
            layernorm(x, rows)
            R = mann_core(x, rows, nmT[0], nmT[1], newmem)
            nc.vector.tensor_add(x[:rows, :], R[:rows, :], x[:rows, :])

        # episodes: rows [55e, 55e+55) of q_out; qn0 holds 0:128, qn1 128:220
        for ep in range(EPC):
            a0, a1 = ep * QPE, (ep + 1) * QPE
            if a1 <= 128 and a0 == 0:
                qv = qn[0]
            else:
                qsc = work.tile([QPE, H], F32, tag="qsc")
                pieces = []
                if a0 < 128:
                    pieces.append((qn[0], a0, min(a1, 128) - a0, 0))
                if a1 > 128:
                    s = max(a0, 128)
                    pieces.append((qn[1], s - 128, a1 - s, max(0, 128 - a0)))
                for (srct, off, cnt, dst_off) in pieces:
                    nc.gpsimd.dma_start(
                        out=qsc[dst_off:dst_off + cnt, :], in_=srct[off:off + cnt, :]
                    )
                qv = qsc
            q2e = small.tile([QPE, 1], F32, tag="q2e")
            qsq = single.tile([QPE, H], F32, tag="qsq")
            nc.scalar.activation(
                out=qsq[:, :], in_=qv[:QPE, :], func=ACT.Square,
                accum_out=q2e[:, :],
            )
            episode_out(qv, q2e, ep)

    return nc


_BUILT = None


def _get_nc():
    global _BUILT
    if _BUILT is None:
        nc = bacc.Bacc(num_devices=NCORES)
        _build(nc)
        if not nc.is_finalized():
            nc.finalize()
        _BUILT = nc
    return _BUILT


def _pool_matrix():
    pm = np.zeros((SUP, EPC * NCLS), np.float32)
    for g in range(EPC * NCLS):
        pm[g * SHOT:(g + 1) * SHOT, g] = 1.0 / SHOT
    return pm


def _shard_inputs(support, query, gamma, beta, mem):
    memT = np.ascontiguousarray(mem.T)
    pm = _pool_matrix()
    gam = np.ascontiguousarray(gamma)
    bet = np.ascontiguousarray(beta)
    in_maps = []
    for c in range(NCORES):
        in_maps.append(
            {
                "sup": np.ascontiguousarray(support[c * SUP:(c + 1) * SUP]),
                "sup1": np.ascontiguousarray(support[c * S16:(c + 1) * S16]),
                "qry": np.ascontiguousarray(query[c * QRY:(c + 1) * QRY]),
                "memt": memT,
                "memb": np.ascontiguousarray(mem),
                "gamma": gam,
                "beta": bet,
                "poolm": pm,
            }
        )
    return in_maps


def run(inputs, trace=False, trace_cores=None):
    support = np.asarray(inputs["support"], dtype=np.float32)
    query = np.asarray(inputs["query"], dtype=np.float32)
    gamma = np.asarray(inputs["gamma"], dtype=np.float32)
    beta = np.asarray(inputs["beta"], dtype=np.float32)
    mem = np.asarray(inputs["mem"], dtype=np.float32)
    assert support.shape == (NCORES * SUP, L, H), support.shape
    assert query.shape == (NCORES * QRY, L, H), query.shape
    nc = _get_nc()
    in_maps = _shard_inputs(support, query, gamma, beta, mem)
    res = run_bass_kernel_spmd(
        nc, in_maps, list(range(NCORES)), trace=trace, trace_cores=trace_cores
    )
    logits = np.concatenate(
        [res.results[c]["logits_o"].reshape(EPC, QPE, NCLS + 1) for c in range(NCORES)],
        axis=0,
    )
    pred = np.concatenate(
        [res.results[c]["pred_o"].reshape(-1) for c in range(NCORES)]
    ).astype(np.int32)
    return (logits, pred), res


def kernel(**inputs):
    (logits, pred), _ = run(inputs)
    return logits, pred


# revision 18
# speedup vs baseline: 1.3126x; 1.1087x over previous
"""Trainium2 Bass kernel for nn_MANN_23965917511952 (8-core SPMD).

Math notes (verified against the jax reference):
  - n = min(B, M) = 128 = M, so the sorted scatter overwrites EVERY memory
    slot: new_mem is just a row-permutation of R_t[:128] from the support
    pass.  softmax(q @ mem^T) @ mem is invariant under row permutation of
    mem, so new_mem can be used in any row order.
  - The updated mem_weight is only consumed by the (discarded) query-pass
    write-back, so argsort/suffix-cumsum/weight softmax are dead code.

Per-core sharding (8 cores):
  - support rows [200c, 200c+200), query rows [220c, 220c+220)  (4 episodes)
  - support rows [16c, 16c+16) (slice of rows 0:128) -> local R16 rows of the
    new memory; a 16KB AllGather replicates the full 128x256 new memory.

Maxpool streaming layout: every big load fills all 128 SBUF partitions
(partition-starved DMAs measured ~1.6x slower).  A "unit" is one (row,
L-half) pair = 64*256 floats = 64KB contiguous in DRAM; a full load is 128
consecutive units as [128, 64, 256] with 64KB-per-partition descriptors.
The in-tile max tree collapses each load to per-unit maxima [128, 256];
per-row maxima are then regrouped via a small DRAM bounce (partition-pair
reduction is impossible on the per-lane compute engines).

Output: logits [32,55,11] f32 and pred [1760] int32 (device computes both;
host only concatenates shards).
"""

import os
import sys

for _p in ("/opt/trn_rl_repo",):
    if os.path.isdir(_p) and _p not in sys.path:
        sys.path.insert(0, _p)

import numpy as np

import concourse.bass as bass
import concourse.tile as tile
from concourse import bacc, masks, mybir
from concourse.bass_utils import run_bass_kernel_spmd

F32 = mybir.dt.float32
AOT = mybir.AluOpType
AX = mybir.AxisListType
ACT = mybir.ActivationFunctionType

NCORES = 8
SUP, QRY = 200, 220          # rows per core
L, H, M = 128, 256, 128      # seq len, hidden, memory slots
S16 = 16                     # rows of support[0:128] owned per core
EPC = 4                      # episodes per core
NCLS, QPE, SHOT = 10, 55, 5  # classes, queries/episode, shots
UNIT = (L // 2) * H          # one (row, L-half) unit: 16384 f32 = 64KB
SUP_BLOCKS = [(0, 128), (128, 200)]
QRY_BLOCKS = [(0, 128), (128, 220)]


def _build(nc: bass.Bass):
    sup = nc.dram_tensor("sup", [SUP, L, H], F32, kind="ExternalInput")
    sup1 = nc.dram_tensor("sup1", [S16, L, H], F32, kind="ExternalInput")
    qry = nc.dram_tensor("qry", [QRY, L, H], F32, kind="ExternalInput")
    memt = nc.dram_tensor("memt", [H, M], F32, kind="ExternalInput")
    memb = nc.dram_tensor("memb", [M, H], F32, kind="ExternalInput")
    gamma = nc.dram_tensor("gamma", [H], F32, kind="ExternalInput")
    beta = nc.dram_tensor("beta", [H], F32, kind="ExternalInput")
    poolm = nc.dram_tensor("poolm", [SUP, EPC * NCLS], F32, kind="ExternalInput")
    logits_o = nc.dram_tensor(
        "logits_o", [EPC * QPE, NCLS + 1], F32, kind="ExternalOutput"
    )
    pred_o = nc.dram_tensor("pred_o", [EPC * QPE, 1], mybir.dt.int32, kind="ExternalOutput")

    from contextlib import ExitStack

    with tile.TileContext(nc) as tc, ExitStack() as ctx:
        chunkp = ctx.enter_context(tc.tile_pool(name="chunkp", bufs=2))
        persist = ctx.enter_context(tc.tile_pool(name="persist", bufs=1))
        single = ctx.enter_context(tc.tile_pool(name="single", bufs=1))
        work = ctx.enter_context(tc.tile_pool(name="work", bufs=2))
        small = ctx.enter_context(tc.tile_pool(name="small", bufs=4))
        psum2 = ctx.enter_context(tc.tile_pool(name="psum2", bufs=2, space="PSUM"))
        psum1 = ctx.enter_context(tc.tile_pool(name="psum1", bufs=1, space="PSUM"))
        dram = ctx.enter_context(tc.tile_pool(name="dram", bufs=1, space="DRAM"))

        # ---------------- constants ----------------
        ident = persist.tile([128, 128], F32, tag="ident")
        masks.make_identity(nc, ident[:])
        epst = persist.tile([128, 1], F32, tag="epst")
        nc.gpsimd.memset(epst[:], 1e-5)
        ones1 = persist.tile([1, 128], F32, tag="ones1")
        nc.gpsimd.memset(ones1[:], 1.0)

        gab = persist.tile([128, H], F32, tag="gab")
        beb = persist.tile([128, H], F32, tag="beb")
        g_ap = gamma[:]
        nc.gpsimd.dma_start(
            out=gab[:],
            in_=bass.AP(tensor=g_ap.tensor, offset=g_ap.offset, ap=[[0, 128]] + list(g_ap.ap)),
        )
        b_ap = beta[:]
        nc.gpsimd.dma_start(
            out=beb[:],
            in_=bass.AP(tensor=b_ap.tensor, offset=b_ap.offset, ap=[[0, 128]] + list(b_ap.ap)),
        )
        memT0 = persist.tile([128, M], F32, tag="memT0")
        memT1 = persist.tile([128, M], F32, tag="memT1")
        nc.gpsimd.dma_start(out=memT0[:], in_=memt[0:128, :])
        nc.gpsimd.dma_start(out=memT1[:], in_=memt[128:256, :])
        memb_sb = persist.tile([M, H], F32, tag="membsb")
        nc.gpsimd.dma_start(out=memb_sb[:], in_=memb[:, :])
        poolm0 = persist.tile([128, EPC * NCLS], F32, tag="poolm0")
        poolm1 = persist.tile([SUP - 128, EPC * NCLS], F32, tag="poolm1")
        nc.gpsimd.dma_start(out=poolm0[:], in_=poolm[0:128, :])
        nc.gpsimd.dma_start(out=poolm1[:], in_=poolm[128:SUP, :])

        # ---------------- helpers ----------------
        def tree_max(t, parts, width):
            """In-place halving max over axis 1 of t[:parts, :width, :]."""
            w = width
            while w > 1:
                nc.vector.tensor_max(
                    t[:parts, : w // 2, :], t[:parts, : w // 2, :], t[:parts, w // 2: w, :]
                )
                w //= 2

        def layernorm(x, rows):
            stats = small.tile([128, 6], F32, tag="stats")
            mv = small.tile([128, 2], F32, tag="mv")
            nc.vector.bn_stats(out=stats[:rows], in_=x[:rows, :])
            nc.vector.bn_aggr(out=mv[:rows], in_=stats[:rows])
            rstd = small.tile([128, 1], F32, tag="rstd")
            nc.scalar.activation(
                out=rstd[:rows], in_=mv[:rows, 1:2], func=ACT.Sqrt,
                bias=epst[:rows], scale=1.0,
            )
            nc.vector.reciprocal(out=rstd[:rows], in_=rstd[:rows])
            nc.vector.tensor_scalar(
                out=x[:rows, :], in0=x[:rows, :],
                scalar1=mv[:rows, 0:1], scalar2=rstd[:rows],
                op0=AOT.subtract, op1=AOT.mult,
            )
            nc.vector.tensor_mul(x[:rows, :], x[:rows, :], gab[:rows, :])
            nc.vector.tensor_add(x[:rows, :], x[:rows, :], beb[:rows, :])

        def mann_core(x, rows, rhs0, rhs1, bank):
            """R = softmax(x @ bank^T) @ bank; returns R in PSUM [rows, H]."""
            xT = []
            for k in range(2):
                tp = psum2.tile([128, 128], F32, tag="tr")
                nc.tensor.transpose(
                    tp[:, :rows], x[:rows, k * 128:(k + 1) * 128], ident[:rows, :rows]
                )
                xk = work.tile([128, 128], F32, tag=f"xT{k}")
                nc.scalar.copy(out=xk[:, :rows], in_=tp[:, :rows])
                xT.append(xk)
            G = psum1.tile([128, M], F32, tag="G")
            nc.tensor.matmul(G[:rows, :], xT[0][:, :rows], rhs0[:, :], start=True, stop=False)
            nc.tensor.matmul(G[:rows, :], xT[1][:, :rows], rhs1[:, :], start=False, stop=True)
            nmax = small.tile([128, 1], F32, tag="nmax")
            nc.vector.tensor_reduce(
                out=nmax[:rows], in_=G[:rows, :], op=AOT.max, axis=AX.X, negate=True
            )
            W = work.tile([128, M], F32, tag="W")
            ssum = small.tile([128, 1], F32, tag="ssum")
            nc.scalar.activation(
                out=W[:rows, :], in_=G[:rows, :], func=ACT.Exp,
                bias=nmax[:rows], scale=1.0, accum_out=ssum[:rows],
            )
            nc.vector.reciprocal(out=ssum[:rows], in_=ssum[:rows])
            nc.vector.tensor_scalar_mul(out=W[:rows, :], in0=W[:rows, :], scalar1=ssum[:rows])
            tpw = psum2.tile([128, 128], F32, tag="tr")
            nc.tensor.transpose(tpw[:, :rows], W[:rows, :], ident[:rows, :rows])
            WT = work.tile([128, 128], F32, tag="WT")
            nc.scalar.copy(out=WT[:, :rows], in_=tpw[:, :rows])
            R = psum2.tile([128, H], F32, tag="R")
            nc.tensor.matmul(R[:rows, :], WT[:, :rows], bank[:, :], start=True, stop=True)
            return R

        # ---------------- sup1 -> new memory (starts first) ----------------
        s1 = chunkp.tile([128, S16, H], F32, tag="chunk")
        ap1 = sup1[:, :, :]
        s1src = bass.AP(
            tensor=ap1.tensor, offset=ap1.offset,
            ap=[[L * H // 8, 128], [H, S16], [1, H]],
        )
        nc.sync.dma_start(out=s1[:, :, :], in_=s1src)
        tree_max(s1, 128, S16)
        # partition regroup (r a) -> r via a DRAM bounce
        scr1 = dram.tile([128, H], F32, tag="scr1")
        nc.gpsimd.dma_start(out=scr1[:, :], in_=s1[:, 0, :])
        s1b = single.tile([S16, 8, H], F32, tag="s1b")
        nc.gpsimd.dma_start(
            out=s1b[:, :, :], in_=scr1[:, :].rearrange("(r a) h -> r a h", a=8)
        )
        tree_max(s1b, S16, 8)
        s16t = single.tile([S16, H], F32, tag="s16t")
        nc.vector.tensor_copy(s16t[:, :], s1b[:, 0, :])
        layernorm(s16t, S16)
        R16 = mann_core(s16t, S16, memT0, memT1, memb_sb)
        r16sb = single.tile([S16, H], F32, tag="r16sb")
        nc.scalar.copy(out=r16sb[:, :], in_=R16[:S16, :])
        cc_in = dram.tile([S16, H], F32, tag="ccin")
        cc_out = dram.tile([M, H], F32, tag="ccout")
        nc.gpsimd.dma_start(out=cc_in[:, :], in_=r16sb[:, :])
        nc.gpsimd.collective_compute(
            "AllGather",
            AOT.bypass,
            replica_groups=[list(range(NCORES))],
            ins=[cc_in.opt()],
            outs=[cc_out.opt()],
        )
        # (newmem is loaded back right before the query blocks, so its
        # collective wait doesn't stall the engine queues during streaming)

        # ---------------- streaming maxpool (unit-packed full loads) -------
        # scr_* hold per-unit maxima: unit u = (row u//2, L-half u%2).
        scr_sup = dram.tile([384, H], F32, tag="scr_sup")
        scr_sup_t = dram.tile([32, H], F32, tag="scr_sup_t")    # 8 rows x G=4
        scr_qry = dram.tile([384, H], F32, tag="scr_qry")
        scr_qry_t = dram.tile([112, H], F32, tag="scr_qry_t")   # 28 rows x G=4

        def stream_full(src_handle, j, scr):
            """Load units [128j, 128j+128) as [128, 64, H], reduce, park."""
            t = chunkp.tile([128, L // 2, H], F32, tag="chunk")
            a = src_handle[:, :, :]
            src = bass.AP(
                tensor=a.tensor, offset=a.offset + j * 128 * UNIT,
                ap=[[UNIT, 128], [H, L // 2], [1, H]],
            )
            nc.sync.dma_start(out=t[:, :, :], in_=src)
            tree_max(t, 128, L // 2)
            nc.scalar.dma_start(out=scr[j * 128:(j + 1) * 128, :], in_=t[:, 0, :])

        def stream_tail(src_handle, row0, rows, g, scr):
            """Tail rows packed (r, lh=g) -> rows*g partitions, L//g slices."""
            parts = rows * g
            ll = L // g
            t = chunkp.tile([128, L // 2, H], F32, tag="chunk")
            a = src_handle[:, :, :]
            src = bass.AP(
                tensor=a.tensor, offset=a.offset + row0 * L * H,
                ap=[[ll * H, parts], [H, ll], [1, H]],
            )
            nc.sync.dma_start(out=t[:parts, :ll, :], in_=src)
            w = ll
            while w > 1:
                nc.vector.tensor_max(
                    t[:parts, : w // 2, :], t[:parts, : w // 2, :], t[:parts, w // 2: w, :]
                )
                w //= 2
            nc.scalar.dma_start(out=scr[0:parts, :], in_=t[:parts, 0, :])

        for j in range(3):
            stream_full(sup, j, scr_sup)
        stream_tail(sup, 192, 8, 4, scr_sup_t)
        for j in range(3):
            stream_full(qry, j, scr_qry)
        stream_tail(qry, 192, 28, 4, scr_qry_t)

        def combine_full(scr, u0, rows_out, dst):
            """dst[:rows_out] = max over the 2 halves; units u0:u0+2*rows."""
            sb = single.tile([128, 2, H], F32, tag="comb")
            nc.scalar.dma_start(
                out=sb[:rows_out, :, :],
                in_=scr[u0:u0 + 2 * rows_out, :].rearrange("(r g) h -> r g h", g=2),
            )
            nc.vector.tensor_max(dst[:rows_out, :], sb[:rows_out, 0, :], sb[:rows_out, 1, :])

        def combine_tail(scr, rows, g, dst, dst_off):
            sb = single.tile([28, 4, H], F32, tag="combt")
            nc.scalar.dma_start(
                out=sb[:rows, :g, :],
                in_=scr[0:rows * g, :].rearrange("(r g) h -> r g h", g=g),
            )
            w = g
            while w > 1:
                nc.vector.tensor_max(
                    sb[:rows, : w // 2, :], sb[:rows, : w // 2, :], sb[:rows, w // 2: w, :]
                )
                w //= 2
            # rebase to dst rows [dst_off, dst_off+rows) (partition shift)
            nc.scalar.dma_start(
                out=dst[dst_off:dst_off + rows, :], in_=sb[:rows, 0, :]
            )

        # ---------------- support blocks ----------------
        proto = psum1.tile([EPC * NCLS, H], F32, tag="proto")
        sn = []
        for b, (r0, r1) in enumerate(SUP_BLOCKS):
            rows = r1 - r0
            x = persist.tile([128, H], F32, tag=f"sn{b}")
            sn.append(x)
            if b == 0:
                combine_full(scr_sup, 0, 128, x)
            else:
                combine_full(scr_sup, 256, 64, x)
                combine_tail(scr_sup_t, 8, 4, x, 64)
            layernorm(x, rows)
            R = mann_core(x, rows, memT0, memT1, memb_sb)
            nc.vector.tensor_add(x[:rows, :], R[:rows, :], x[:rows, :])
            pm = poolm0 if b == 0 else poolm1
            nc.tensor.matmul(
                proto[:, :], pm[:rows, :], x[:rows, :],
                start=(b == 0), stop=(b == len(SUP_BLOCKS) - 1),
            )

        NP = EPC * NCLS  # 40
        proto_sb = persist.tile([NP, H], F32, tag="protosb")
        nc.scalar.copy(out=proto_sb[:, :], in_=proto[:, :])
        p2 = persist.tile([NP, 1], F32, tag="p2")
        psq = single.tile([NP, H], F32, tag="psq")
        nc.scalar.activation(
            out=psq[:, :], in_=proto_sb[:, :], func=ACT.Square, accum_out=p2[:, :]
        )
        pT = []
        for k in range(2):
            tp = psum2.tile([128, 128], F32, tag="tr")
            nc.tensor.transpose(
                tp[:, :NP], proto_sb[:NP, k * 128:(k + 1) * 128], ident[:NP, :NP]
            )
            pk = persist.tile([128, NP], F32, tag=f"pT{k}")
            nc.scalar.mul(out=pk[:, :], in_=tp[:, :NP], mul=2.0)
            pT.append(pk)
        tpp = psum2.tile([128, 128], F32, tag="tr")
        nc.tensor.transpose(tpp[:1, :NP], p2[:NP, 0:1], ident[:NP, :NP])
        negp2 = persist.tile([1, NP], F32, tag="negp2")
        nc.scalar.mul(out=negp2[:, :], in_=tpp[:1, :NP], mul=-1.0)

        # ---------------- new memory for the query pass ----------------
        newmem = persist.tile([M, H], F32, tag="newmem")
        nc.gpsimd.dma_start(out=newmem[:, :], in_=cc_out[:, :])
        nmT = []
        for k in range(2):
            tp = psum2.tile([128, 128], F32, tag="tr")
            nc.tensor.transpose(tp[:, :], newmem[:, k * 128:(k + 1) * 128], ident[:, :])
            nk = persist.tile([128, 128], F32, tag=f"nmT{k}")
            nc.scalar.copy(out=nk[:, :], in_=tp[:, :])
            nmT.append(nk)

        # ---------------- query blocks + episodes ----------------
        def episode_out(qv, q2e, ep):
            qT = []
            for k in range(2):
                tp = psum2.tile([128, 128], F32, tag="tr")
                nc.tensor.transpose(
                    tp[:, :QPE], qv[:QPE, k * 128:(k + 1) * 128],
                    ident[:QPE, :QPE],
                )
                qk = work.tile([128, QPE], F32, tag=f"qT{k}")
                nc.scalar.copy(out=qk[:, :], in_=tp[:, :QPE])
                qT.append(qk)
            dot = psum1.tile([QPE, 16], F32, tag="dot")
            nc.tensor.matmul(
                dot[:, :NCLS], qT[0][:, :], pT[0][:, ep * NCLS:(ep + 1) * NCLS],
                start=True, stop=False,
            )
            nc.tensor.matmul(
                dot[:, :NCLS], qT[1][:, :], pT[1][:, ep * NCLS:(ep + 1) * NCLS],
                start=False, stop=False,
            )
            nc.tensor.matmul(
                dot[:, :NCLS], ones1[:1, :QPE], negp2[:1, ep * NCLS:(ep + 1) * NCLS],
                start=False, stop=True,
            )
            lf = work.tile([QPE, 16], F32, tag="lf")
            nc.vector.tensor_scalar(
                out=lf[:, 0:NCLS], in0=dot[:, :NCLS],
                scalar1=q2e[:QPE], scalar2=None, op0=AOT.subtract,
            )
            mn = small.tile([QPE, 1], F32, tag="mn")
            nc.vector.tensor_reduce(out=mn[:, :], in_=lf[:, 0:NCLS], op=AOT.min, axis=AX.X)
            nc.vector.tensor_scalar_add(out=lf[:, NCLS:NCLS + 1], in0=mn[:, :], scalar1=-1.0)
            mx8 = small.tile([QPE, 8], F32, tag="mx8")
            nc.vector.max(out=mx8[:, :], in_=lf[:, 0:NCLS + 1])
            idx8 = small.tile([QPE, 8], mybir.dt.uint32, tag="idx8")
            nc.vector.max_index(out=idx8[:, :], in_max=mx8[:, :], in_values=lf[:, 0:NCLS + 1])
            nc.gpsimd.dma_start(
                out=logits_o[ep * QPE:(ep + 1) * QPE, :], in_=lf[:, 0:NCLS + 1]
            )
            nc.gpsimd.dma_start(
                out=pred_o[ep * QPE:(ep + 1) * QPE, :],
                in_=idx8[:, 0:1].bitcast(mybir.dt.int32),
            )

        qn = []
        for qb, (r0, r1) in enumerate(QRY_BLOCKS):
            rows = r1 - r0
            x = persist.tile([128, H], F32, tag=f"qn{qb}")
            qn.append(x)
            if qb == 0:
                combine_full(scr_qry, 0, 128, x)
            else:
                combine_full(scr_qry, 256, 64, x)
                combine_tail(scr_qry_t, 28, 4, x, 64)
            layernorm(x, rows)
            R = mann_core(x, rows, nmT[0], nmT[1], newmem)
            nc.vector.tensor_add(x[:rows, :], R[:rows, :], x[:rows, :])

        # episodes: rows [55e, 55e+55) of q_out; qn0 holds 0:128, qn1 128:220
        for ep in range(EPC):
            a0, a1 = ep * QPE, (ep + 1) * QPE
            if a1 <= 128 and a0 == 0:
                qv = qn[0]
            else:
                qsc = work.tile([QPE, H], F32, tag="qsc")
                pieces = []
                if a0 < 128:
                    pieces.append((qn[0], a0, min(a1, 128) - a0, 0))
                if a1 > 128:
                    s = max(a0, 128)
                    pieces.append((qn[1], s - 128, a1 - s, max(0, 128 - a0)))
                for (srct, off, cnt, dst_off) in pieces:
                    nc.scalar.dma_start(
                        out=qsc[dst_off:dst_off + cnt, :], in_=srct[off:off + cnt, :]
                    )
                qv = qsc
            q2e = small.tile([QPE, 1], F32, tag="q2e")
            qsq = single.tile([QPE, H], F32, tag="qsq")
            nc.scalar.activation(
                out=qsq[:, :], in_=qv[:QPE, :], func=ACT.Square,
                accum_out=q2e[:, :],
            )
            episode_out(qv, q2e, ep)

    return nc


_BUILT = None


def _get_nc():
    global _BUILT
    if _BUILT is None:
        nc = bacc.Bacc(num_devices=NCORES)
        _build(nc)
        if not nc.is_finalized():
            nc.finalize()
        _BUILT = nc
    return _BUILT


def _pool_matrix():
    pm = np.zeros((SUP, EPC * NCLS), np.float32)
    for g in range(EPC * NCLS):
        pm[g * SHOT:(g + 1) * SHOT, g] = 1.0 / SHOT
    return pm


def _shard_inputs(support, query, gamma, beta, mem):
    memT = np.ascontiguousarray(mem.T)
    pm = _pool_matrix()
    gam = np.ascontiguousarray(gamma)
    bet = np.ascontiguousarray(beta)
    in_maps = []
    for c in range(NCORES):
        in_maps.append(
            {
                "sup": np.ascontiguousarray(support[c * SUP:(c + 1) * SUP]),
                "sup1": np.ascontiguousarray(support[c * S16:(c + 1) * S16]),
                "qry": np.ascontiguousarray(query[c * QRY:(c + 1) * QRY]),
                "memt": memT,
                "memb": np.ascontiguousarray(mem),
                "gamma": gam,
                "beta": bet,
                "poolm": pm,
            }
        )
    return in_maps


def run(inputs, trace=False, trace_cores=None):
    support = np.asarray(inputs["support"], dtype=np.float32)
    query = np.asarray(inputs["query"], dtype=np.float32)
    gamma = np.asarray(inputs["gamma"], dtype=np.float32)
    beta = np.asarray(inputs["beta"], dtype=np.float32)
    mem = np.asarray(inputs["mem"], dtype=np.float32)
    assert support.shape == (NCORES * SUP, L, H), support.shape
    assert query.shape == (NCORES * QRY, L, H), query.shape
    nc = _get_nc()
    in_maps = _shard_inputs(support, query, gamma, beta, mem)
    res = run_bass_kernel_spmd(
        nc, in_maps, list(range(NCORES)), trace=trace, trace_cores=trace_cores
    )
    logits = np.concatenate(
        [res.results[c]["logits_o"].reshape(EPC, QPE, NCLS + 1) for c in range(NCORES)],
        axis=0,
    )
    pred = np.concatenate(
        [res.results[c]["pred_o"].reshape(-1) for c in range(NCORES)]
    ).astype(np.int32)
    return (logits, pred), res


def kernel(**inputs):
    (logits, pred), _ = run(inputs)
    return logits, pred
